# revision 4
# baseline (speedup 1.0000x reference)
"""Low-pass FFT filtering kernel for Trainium2 (8 NeuronCores) — v2.

Math: per (batch b, channel i), X = x[b,:,:,i] (256x256):
    out_i = P @ X_i + X_i @ P,  P = W @ W.T,  W [256, 31] orthonormal
    (modes 0..15 of the ortho rfft).  out = W C + D^T W^T with
    C = W^T X  [31, n],  D = W^T X^T  [31, m].

Schedule notes: the kernel is jointly DMA-bound (12.6 MB HBM traffic)
and PE-bound (49152 streamed matmul columns).  Every input DMA is one
contiguous [128, F] HBM block aligned to the chunk that consumes it
(the Tile scheduler merges semaphore waits to the ring-FIFO max of an
instruction's deps — misaligned transfers made matmuls wait on later
unrelated DMAs).  Emission-order rule: each engine's stream holds
only true dependencies in consumption order — Act carries ONLY the
C/D copies, emitted AFTER the ph2 block that precedes their consumer
(so ph2(g-1)'s merged Act wait is exactly D(g-1), not C(g)); DVE
carries ONLY the output casts.  Outputs ride the software DGE ring in
paired blocks behind the input stream; the final chunk ships in fine
pieces to shorten the terminal chain.
Sharding: batch b -> core b (8 cores, no communication).
"""

import os
import sys
import types

import numpy as np

import concourse.bass as bass
import concourse.bacc as bacc
import concourse.tile as tile
from concourse import mybir
from concourse.bass_utils import run_bass_kernel_spmd

B, M, N, I = 8, 256, 256, 32
KMAX = 16           # modes kept: 0..15
R = 2 * KMAX - 1    # 31 real basis vectors
FREE = I * N        # 8192
NCHUNK = 8
CW = FREE // NCHUNK      # 1024 cols = 4 channels per chunk
CH_PER_CHUNK = I // NCHUNK
F32 = mybir.dt.float32
F16 = mybir.dt.float16
NPDT = np.float16

LAST_RESULTS = None


def _ensure_ntff_hook():
    try:
        from antenv.axon_hooks import get_axon_ntff_profile_hook  # noqa: F401
        return
    except ImportError:
        pass
    try:
        from trn_agent_boot.trn_boot import _ntff_profile_via_ctypes
        hook = _ntff_profile_via_ctypes("/opt/axon/libaxon_pjrt.so")
    except Exception:
        hook = None
    mod = types.ModuleType("antenv.axon_hooks")
    _state = {"hook": hook}
    mod.get_axon_ntff_profile_hook = lambda: _state["hook"]
    mod.set_axon_ntff_profile_hook = lambda h: _state.update(hook=h)
    sys.modules["antenv.axon_hooks"] = mod
    try:
        import antenv
        antenv.axon_hooks = mod
    except ImportError:
        pass


def _basis():
    t = np.arange(N)
    cols = [np.ones(N) / np.sqrt(N)]
    for k in range(1, KMAX):
        cols.append(np.sqrt(2.0 / N) * np.cos(2 * np.pi * k * t / N))
        cols.append(-np.sqrt(2.0 / N) * np.sin(2 * np.pi * k * t / N))
    return np.stack(cols, axis=1).astype(np.float32)  # [256, 31]


def _build_nc():
    nc = bacc.Bacc("TRN2", target_bir_lowering=False, debug=False,
                   enable_asserts=False, num_devices=8,
                   enable_partition_id=False)

    # xa/xt: chunk-blocked [128, 16384]; chunk g cols [2048] =
    # [top half rows | bottom half rows] of the m-major / n-major view.
    xa = nc.declare_dram_parameter("xa", [128, 62 + 2 * FREE], F16,
                                   isOutput=False)
    xt = nc.declare_dram_parameter("xt", [128, 2 * FREE], F16, isOutput=False)
    wz = nc.declare_dram_parameter("wz", [R + 1, CW], F16, isOutput=False)
    zw = nc.declare_dram_parameter("zw", [R + 1, CW], F16, isOutput=False)
    out = nc.declare_dram_parameter("out", [128, 2 * FREE], F16, isOutput=True)

    with tile.TileContext(nc) as tc:
        with tc.tile_pool(name="all", bufs=1) as pool, \
             tc.tile_pool(name="ps", bufs=1, space=bass.MemorySpace.PSUM) as psp:
            xasb = pool.tile([128, 62 + 2 * FREE], F16, name="xasb")
            xtsb = pool.tile([128, 2 * FREE], F16, name="xtsb")
            otsb = pool.tile([128, 2 * FREE], F16, name="otsb")
            Lt = [pool.tile([63, CW], F16, name=f"L{j}") for j in range(2)]
            Rt = [pool.tile([63, CW], F16, name=f"R{j}") for j in range(2)]
            pcd = [psp.tile([63, CW], F32, name=f"pcd{j}") for j in range(2)]
            p2 = [psp.tile([128, CW], F32, name=f"p2{j}") for j in range(2)]

            W0 = xasb[:, 0:R]
            W1 = xasb[:, R:2 * R]

            # --- input DMA issues, chunk-aligned ---
            nc.sync.dma_start(out=xasb[:, 0:62 + 1024], in_=xa[:, 0:62 + 1024])
            nc.sync.dma_start(out=xasb[:, 62 + 1024:62 + 2048],
                              in_=xa[:, 62 + 1024:62 + 2048])
            nc.scalar.dma_start(out=xtsb[:, 0:1024], in_=xt[:, 0:1024])
            nc.scalar.dma_start(out=xtsb[:, 1024:2048], in_=xt[:, 1024:2048])
            for g in range(1, NCHUNK):
                gsl = slice(62 + 2 * g * CW, 62 + 2 * (g + 1) * CW)
                nc.sync.dma_start(out=xasb[:, gsl], in_=xa[:, gsl])
            for j in range(2):
                nc.gpsimd.dma_start(out=Lt[j][0:32, :], in_=wz[:])
                nc.gpsimd.dma_start(out=Rt[j][31:63, :], in_=zw[:])
            for g in range(1, 3):
                gsl = slice(2 * g * CW, 2 * (g + 1) * CW)
                nc.scalar.dma_start(out=xtsb[:, gsl], in_=xt[:, gsl])

            for g in range(NCHUNK):
                Lg = Lt[g % 2]
                Rg = Rt[g % 2]
                pg = pcd[g % 2]
                c0 = 62 + 2 * g * CW
                c1 = c0 + CW
                t0 = 2 * g * CW
                t1 = t0 + CW

                # p-state keep-alive: the first two chunks of the loop are
                # gated on the DMA ramp; dummy matmuls on chunk-0 data keep
                # the PE continuously busy through the data gap so the HAM
                # clock ramp (needs 3us gapless) is not reset.  The written
                # pcd region is immediately reset by this chunk's real
                # phase-1 (start=True).
                nfill = {1: 13, 2: 6}.get(g, 0)
                for ff in range(nfill):
                    nc.tensor.matmul(
                        pg[0:R, 0:512], W0,
                        xasb[:, 62 + (ff % 2) * 512:62 + (ff % 2) * 512 + 512],
                        start=True, stop=True, tile_position=(0, 0))

                if g + 3 < NCHUNK:
                    gsl = slice(2 * (g + 3) * CW, 2 * (g + 4) * CW)
                    nc.scalar.dma_start(out=xtsb[:, gsl], in_=xt[:, gsl])
                # ship finished output chunk pairs on the SWDGE ring
                if g >= 3 and g % 2 == 1:
                    osl = slice(2 * (g - 3) * CW, 2 * (g - 1) * CW)
                    nc.gpsimd.dma_start(out=out[:, osl], in_=otsb[:, osl])

                # phase 1: C = W^T X (rows 0:31), D = W^T X^T (rows 32:63);
                # 512-col matmuls (PSUM bank limit), K halves accumulate,
                # grouped by weight half.
                for rows, src, b0, b1, cpos in (
                        (slice(0, R), xasb, c0, c1, 0),
                        (slice(32, 63), xtsb, t0, t1, 32)):
                    for h, Wh, base in ((0, W0, b0), (1, W1, b1)):
                        for f in range(2):
                            fsl = slice(f * 512, (f + 1) * 512)
                            nc.tensor.matmul(
                                pg[rows, fsl], Wh,
                                src[:, base + f * 512:base + (f + 1) * 512],
                                start=(h == 0), stop=(h == 1),
                                tile_position=(0, cpos))

                # phase 2 for the previous chunk, emitted BEFORE this
                # chunk's copies (wait-order hygiene)
                if g > 0:
                    _phase2(nc, otsb, p2, Lt, Rt, g - 1, fine=False)

                # mid-kernel p-state keep-alive: at 2.4GHz the PE outruns
                # the input DMA cadence; pad the pre-ph1(g+1) data gap with
                # dummy matmuls into the pcd buffer ph1(g+1) will reset.
                # Emitted here so the merged Act wait is the already-done
                # copies(g-1), and the pcd WAR partner is the same.
                nfill2 = {3: 4, 4: 4, 5: 4, 6: 4, 7: 4}.get(g + 1, 0)
                ftgt = pcd[(g + 1) % 2]
                if g == NCHUNK - 1:
                    # tail: pad the wait for the final copies; chunk-6's
                    # pcd buffer is retired (its copies are done)
                    nfill2 = 5
                    ftgt = pcd[(g - 1) % 2]
                for ff in range(nfill2):
                    nc.tensor.matmul(
                        ftgt[0:R, 0:512], W0,
                        xasb[:, 62 + (ff % 2) * 512:62 + (ff % 2) * 512 + 512],
                        start=True, stop=True, tile_position=(0, 0))

                if g == NCHUNK - 1:
                    # split the final copies so the tail ph2 starts after
                    # the first halves instead of the full serial pair
                    nc.scalar.copy(Rg[0:R, 0:512], pg[0:R, 0:512])
                    nc.scalar.copy(Lg[32:63, 0:512], pg[32:63, 0:512])
                    nc.scalar.copy(Rg[0:R, 512:CW], pg[0:R, 512:CW])
                    nc.scalar.copy(Lg[32:63, 512:CW], pg[32:63, 512:CW])
                else:
                    nc.scalar.copy(Rg[0:R, :], pg[0:R, :])
                    nc.scalar.copy(Lg[32:63, :], pg[32:63, :])
            _phase2(nc, otsb, p2, Lt, Rt, NCHUNK - 1, fine=True)

            # remaining outputs on the by-now-idle HWDGE rings
            nc.sync.dma_start(out=out[:, 12288:14336],
                              in_=otsb[:, 12288:14336])
            ofine = [(14336, 15360), (15360, 15872), (15872, 16128),
                     (16128, 16384)]
            for k, (lo, hi) in enumerate(ofine):
                eng = nc.scalar if k % 2 == 0 else nc.sync
                eng.dma_start(out=out[:, lo:hi], in_=otsb[:, lo:hi])

    nc.finalize()
    return nc


def _phase2(nc, otsb, p2, Lt, Rt, g, fine):
    """Emit phase-2 matmuls + casts (all on DVE) for chunk g into otsb."""
    Lg = Lt[g % 2]
    Rg = Rt[g % 2]
    for j in range(2):
        pj = p2[j]
        for i in range(CH_PER_CHUNK):
            csl = slice(i * N, (i + 1) * N)
            jsl = slice(i * N + j * 128, i * N + (j + 1) * 128)
            nc.tensor.matmul(pj[:, csl], Lg[:, jsl], Rg[:, csl],
                             start=True, stop=True)
        osl = 2 * g * CW + j * CW
        if fine and j == 1:
            for lo, hi in ((0, 512), (512, 768), (768, 1024)):
                nc.vector.tensor_copy(otsb[:, osl + lo:osl + hi],
                                      pj[:, lo:hi])
        elif fine:
            nc.vector.tensor_copy(otsb[:, osl:osl + 512], pj[:, 0:512])
            nc.vector.tensor_copy(otsb[:, osl + 512:osl + CW],
                                  pj[:, 512:CW])
        else:
            nc.vector.tensor_copy(otsb[:, osl:osl + CW], pj[:])


_NC = None


def kernel(x: np.ndarray) -> np.ndarray:
    global _NC, LAST_RESULTS
    x = np.asarray(x)
    assert x.shape == (B, M, N, I), x.shape

    W = _basis().astype(NPDT)          # [256, 31]
    Wt = W.T.copy()                    # [31, 256]
    w2_np = np.concatenate([W[0:128, :], W[128:256, :]], axis=1)  # [128, 62]
    wtile = np.tile(Wt, (1, CH_PER_CHUNK))                        # [31, 1024]
    wz_np = np.concatenate([wtile, np.zeros((1, CW), NPDT)], axis=0)
    zw_np = np.concatenate([np.zeros((1, CW), NPDT), wtile], axis=0)

    if _NC is None:
        _NC = _build_nc()

    xq = np.asarray(x, dtype=NPDT)
    in_maps = []
    for b in range(B):
        xcm = np.ascontiguousarray(xq[b].transpose(0, 2, 1)).reshape(M, FREE)
        xtm = np.ascontiguousarray(xq[b].transpose(1, 2, 0)).reshape(N, I * M)
        xa_np = np.empty((128, 62 + 2 * FREE), NPDT)
        xt_np = np.empty((128, 2 * FREE), NPDT)
        xa_np[:, 0:62] = w2_np
        for g in range(NCHUNK):
            gsl = slice(g * CW, (g + 1) * CW)
            a0 = 62 + 2 * g * CW
            xa_np[:, a0:a0 + CW] = xcm[0:128, gsl]
            xa_np[:, a0 + CW:a0 + 2 * CW] = xcm[128:256, gsl]
            xt_np[:, 2 * g * CW:2 * g * CW + CW] = xtm[0:128, gsl]
            xt_np[:, 2 * g * CW + CW:2 * (g + 1) * CW] = xtm[128:256, gsl]
        in_maps.append({
            "xa": xa_np, "xt": xt_np,
            "wz": wz_np, "zw": zw_np,
        })

    trace = bool(int(os.environ.get("KERNEL_TRACE", "0")))
    if trace:
        _ensure_ntff_hook()
    last_err = None
    for attempt in range(3):
        try:
            LAST_RESULTS = run_bass_kernel_spmd(_NC, in_maps, list(range(B)),
                                                trace=trace and attempt == 0)
            break
        except Exception as e:
            last_err = e
            import time as _time
            _time.sleep(2.0)
            try:
                import jax
                jax.clear_caches()
                jax.extend.backend.clear_backends()
            except Exception:
                pass
    else:
        raise last_err

    out = np.empty((B, M, N, I), np.float32)
    for b in range(B):
        dev = LAST_RESULTS.results[b]["out"].astype(np.float32)  # [128, 16K]
        om = np.empty((M, FREE), np.float32)
        for g in range(NCHUNK):
            gsl = slice(g * CW, (g + 1) * CW)
            om[0:128, gsl] = dev[:, 2 * g * CW:2 * g * CW + CW]
            om[128:256, gsl] = dev[:, 2 * g * CW + CW:2 * (g + 1) * CW]
        out[b] = om.reshape(M, I, N).transpose(0, 2, 1)
    return out
